# revision 7
# baseline (speedup 1.0000x reference)
"""Trainium2 Bass kernel v2 for nn_Network_5772436046487 (gnn_message_passing).

Recurrence (T=50 steps, B=8, N=50000 nodes, E=1.6M edges):
    v' = v + DT*(-v + bias + scatter_add(w * relu(v)[src], tgt) + x_t)/tau

v1 -> v2 changes, driven by HW ablation (ap_gather has ~40us FIXED cost per
instruction; collective 364us; per-instruction overheads dominate the tail):
  - 6 ap_gathers/step instead of 28: edge streams in 2 chunks of 7168
    (A+B), boundary sampling as 2 full gathers of 6416 from a full-prefix
    scan buffer (no ring).
  - SBUF aliasing: `scratch` [128,14336] holds gather outputs AND boundary
    samples; `scanbuf` [128,14336] holds mul outputs and in-place prefix.
  - update layout [128,400] with partition p = h*8 + b, col c, node
    n = h*400 + c: the 16-partial merge becomes 16 partition-offset
    matmuls (psum[8h:8h+8] = sel^T x diff[:, h*400:+400]); no transposes,
    no ACT copies.
  - single-DMA r staging; Shared-output AllGather.
"""

import os
import sys
import time

os.environ.setdefault("JAX_COMPILATION_CACHE_DIR", "/tmp/jax_cache_gnn")
os.environ.setdefault("JAX_PERSISTENT_CACHE_MIN_COMPILE_TIME_SECS", "2")
os.environ.setdefault("JAX_PERSISTENT_CACHE_MIN_ENTRY_SIZE_BYTES", "0")

for _p in ("/opt/trn_rl_repo", "/root/.axon_site/_ro/trn_rl_repo"):
    if os.path.isdir(_p) and _p not in sys.path:
        sys.path.insert(0, _p)

import numpy as np

N_NODES = 50000
N_EDGES = 1_600_000
T = 50
B = 8
DT = 0.02

NC = 8  # cores
CORE_REAL = 6250  # real nodes per core
CORE_PAD = 6400  # padded nodes per core
SLICE = 3200  # nodes per (g,s) sixteenth slice
NQ = 50  # 128-node windows per core
UPD_COLS = NQ * B  # 400 cols, update layout [128, (q b)]

CH = 6560  # edge chunk (idx per gather)
NCH = 2
STREAM = NCH * CH  # 13120 slots per (g,s) stream (data max: 13103)
BOUND_T = 6416  # boundary samples per stream (1 dummy + 6400 + pad)

_CACHE = {}


def _wrap_idx_groups(idx_by_group):
    """idx_by_group: [8, N] int -> [128, N//16] int16 wrapped per group."""
    G, N = idx_by_group.shape
    assert G == 8 and N % 16 == 0
    out = np.empty((128, N // 16), dtype=np.int16)
    for g in range(8):
        out[16 * g : 16 * g + 16, :] = (
            idx_by_group[g].reshape(N // 16, 16).T.astype(np.int16)
        )
    return out


def _preprocess(x, bias, time_const, sign, syn_count, syn_strength,
                source_idx, target_idx):
    """Host-side graph compilation -> per-core input dicts."""
    from concourse import mybir

    bf16 = mybir.dt.np(mybir.dt.bfloat16)
    tau = np.maximum(time_const.astype(np.float64), DT)
    BC = (DT / tau).astype(np.float64)  # per real node
    A = (1.0 - DT / tau).astype(np.float32)
    weight = (sign.astype(np.float64) * syn_count.astype(np.float64)
              * np.maximum(syn_strength.astype(np.float64), 0.0))

    src = source_idx.astype(np.int64)
    tgt = target_idx.astype(np.int64)

    def pid(n):
        return (n // CORE_REAL) * CORE_PAD + (n % CORE_REAL)

    spid = pid(src)
    tpid = pid(tgt)
    tcore = tpid // CORE_PAD
    tloc = tpid % CORE_PAD
    g = spid // CORE_PAD
    s = (spid % CORE_PAD) // SLICE
    sloc = spid % SLICE
    wprime = (weight * BC[tgt]).astype(np.float32)

    order = np.lexsort((tloc, s, g, tcore))
    spid_s, tcore_s, g_s, s_s = spid[order], tcore[order], g[order], s[order]
    sloc_s, tloc_s, w_s = sloc[order], tloc[order], wprime[order]
    key = ((tcore_s * 8 + g_s) * 2 + s_s)
    starts = np.searchsorted(key, np.arange(NC * 16), side="left")
    ends = np.searchsorted(key, np.arange(NC * 16), side="right")
    maxlen = int((ends - starts).max())
    assert maxlen + 1 <= STREAM, f"stream overflow: {maxlen + 1} > {STREAM}"

    def to_upd_layout(vec_b_n):  # [B, CORE_PAD] -> [128, 400], node = q*128+p
        return (vec_b_n.reshape(B, NQ, 128).transpose(2, 1, 0)
                .reshape(128, UPD_COLS).astype(np.float32))

    per_core = []
    for c in range(NC):
        idx_streams = np.zeros((8, 2, STREAM), dtype=np.int16)
        w_streams = np.zeros((8, 2, STREAM), dtype=np.float32)
        bidx = np.zeros((8, 2, BOUND_T), dtype=np.int64)
        for gg in range(8):
            for ss in range(2):
                k = (c * 8 + gg) * 2 + ss
                a, b_ = int(starts[k]), int(ends[k])
                n = b_ - a
                # position 0 is a dummy edge (idx 0, w 0)
                idx_streams[gg, ss, 1 : n + 1] = sloc_s[a:b_]
                w_streams[gg, ss, 1 : n + 1] = w_s[a:b_]
                # sample positions: col 0 = dummy pos 0; col 1+t = cnt(t)
                cnt = np.searchsorted(tloc_s[a:b_], np.arange(CORE_PAD),
                                      side="right")
                bidx[gg, ss, 1 : 1 + CORE_PAD] = cnt
        assert int(bidx.max()) < STREAM

        idxA = _wrap_idx_groups(idx_streams[:, 0, :])
        idxB = _wrap_idx_groups(idx_streams[:, 1, :])
        bidxA = _wrap_idx_groups(bidx[:, 0, :].astype(np.int16))
        bidxB = _wrap_idx_groups(bidx[:, 1, :].astype(np.int16))
        # weights in partition layout p = 16g + 2b + s
        wq = np.zeros((128, STREAM), dtype=np.float32)
        for gg in range(8):
            for ss in range(2):
                for bb in range(B):
                    wq[16 * gg + 2 * bb + ss] = w_streams[gg, ss]
        wq = wq.astype(bf16)

        n0 = c * CORE_REAL
        sl = slice(n0, n0 + CORE_REAL)

        Ap = np.zeros((B, CORE_PAD), dtype=np.float32)
        Ap[:, :CORE_REAL] = A[sl][None, :]
        v0p = np.zeros((B, CORE_PAD), dtype=np.float32)
        v0p[:, :CORE_REAL] = bias[sl][None, :]
        Tl = x.shape[0]
        xc = np.zeros((Tl, B, CORE_PAD), dtype=np.float32)
        xc[:, :, :CORE_REAL] = (
            BC[sl][None, None, :]
            * (x[:, :, sl].astype(np.float64) + bias[sl][None, None, :])
        ).astype(np.float32)
        xprime = (xc.reshape(Tl, B, NQ, 128).transpose(0, 3, 2, 1)
                  .reshape(Tl, 128, UPD_COLS))

        sel = np.zeros((128, 8), dtype=np.float32)
        for p in range(128):
            sel[p, (p % 16) // 2] = 1.0
        mask = np.zeros((128, 8), dtype=np.uint32)
        mask[1::2, :] = 1  # s=1 partitions (p odd)
        ident = np.eye(128, dtype=np.float32)

        per_core.append(dict(
            wq=wq, idxA=idxA, idxB=idxB, bidxA=bidxA, bidxB=bidxB,
            xprime=np.ascontiguousarray(xprime),
            Ad=to_upd_layout(Ap), v0=to_upd_layout(v0p),
            mask=mask, sel=sel, ident=ident,
        ))
    return per_core


def _build(T_steps, tiny_x=False, shared_out=True):
    import concourse.bacc as bacc
    import concourse.mybir as mybir
    import concourse.tile as tile

    dt = mybir.dt
    AF = mybir.ActivationFunctionType
    OP = mybir.AluOpType
    nc = bacc.Bacc("TRN2", target_bir_lowering=False, debug=False,
                   num_devices=NC)

    wq_d = nc.dram_tensor("wq", [128, STREAM], dt.bfloat16,
                          kind="ExternalInput")
    idxA_d = nc.dram_tensor("idxA", [128, STREAM // 16], dt.int16,
                            kind="ExternalInput")
    idxB_d = nc.dram_tensor("idxB", [128, STREAM // 16], dt.int16,
                            kind="ExternalInput")
    bidxA_d = nc.dram_tensor("bidxA", [128, BOUND_T // 16], dt.int16,
                             kind="ExternalInput")
    bidxB_d = nc.dram_tensor("bidxB", [128, BOUND_T // 16], dt.int16,
                             kind="ExternalInput")
    xprime_d = nc.dram_tensor("xprime",
                              [1 if tiny_x else T_steps, 128, UPD_COLS],
                              dt.float32, kind="ExternalInput")
    Ad_d = nc.dram_tensor("Ad", [128, UPD_COLS], dt.float32,
                          kind="ExternalInput")
    v0_d = nc.dram_tensor("v0", [128, UPD_COLS], dt.float32,
                          kind="ExternalInput")
    mask_d = nc.dram_tensor("mask", [128, 8], dt.uint32, kind="ExternalInput")
    sel_d = nc.dram_tensor("sel", [128, 8], dt.float32, kind="ExternalInput")
    ident_d = nc.dram_tensor("ident", [128, 128], dt.float32,
                             kind="ExternalInput")
    out_d = nc.dram_tensor("vs", [T_steps, 128, UPD_COLS], dt.float32,
                           kind="ExternalOutput")
    r_all_d = nc.dram_tensor("r_all_sh", [NC, B * CORE_PAD], dt.float32,
                             addr_space="Shared" if shared_out else "Local")

    with tile.TileContext(nc) as tc:
        with (
            tc.tile_pool(name="sbuf", bufs=1) as pool,
            tc.tile_pool(name="psum", bufs=2, space="PSUM") as psum_pool,
            tc.tile_pool(name="dram", bufs=1, space="DRAM") as dram_pool,
        ):
            wq = pool.tile_from(wq_d[:])
            idxA = pool.tile_from(idxA_d[:])
            idxB = pool.tile_from(idxB_d[:])
            bidxA = pool.tile_from(bidxA_d[:])
            bidxB = pool.tile_from(bidxB_d[:])
            Ad = pool.tile_from(Ad_d[:])
            mask8 = pool.tile_from(mask_d[:])
            sel = pool.tile_from(sel_d[:])
            ident = pool.tile_from(ident_d[:])
            v = pool.tile_from(v0_d[:])

            r_sb = pool.tile([128, UPD_COLS], dt.float32)
            r_full = pool.tile([128, SLICE], dt.float32)
            scratch = pool.tile([128, STREAM], dt.float32)
            scanbuf = pool.tile([128, STREAM], dt.float32)
            xcur = pool.tile([128, UPD_COLS], dt.float32, tag="xq0")
            xnxt = pool.tile([128, UPD_COLS], dt.float32, tag="xq1")
            t1 = pool.tile([128, UPD_COLS], dt.float32)

            r_own = dram_pool.tile([B, CORE_PAD], dt.float32)

            nc.sync.dma_start(xcur[:], xprime_d[0])

            xt = [xcur, xnxt]
            maskCH = mask8[:, 0:1].broadcast_to([128, CH])
            maskBT = mask8[:, 0:1].broadcast_to([128, BOUND_T])

            for t in range(T_steps):
                # ---- halo exchange of r = relu(v) ----
                nc.scalar.activation(r_sb[:], v[:], AF.Relu)
                for bb in range(B):
                    nc.sync.dma_start(
                        r_own[bb : bb + 1, :].rearrange(
                            "o (q p) -> (o p) q", p=128),
                        r_sb[:, bb :: B],
                    )
                nc.gpsimd.collective_compute(
                    "AllGather", OP.bypass,
                    replica_groups=[list(range(NC))],
                    ins=[r_own[:].opt()], outs=[r_all_d[:].opt()],
                )
                nc.sync.dma_start(
                    r_full[:],
                    r_all_d[:].rearrange("g (b s n) -> (g b s) n", b=B, s=2),
                )
                if t + 1 < T_steps:
                    nc.sync.dma_start(xt[(t + 1) % 2][:],
                                      xprime_d[0 if tiny_x else t + 1])

                # ---- edge phase: 2 chunks x (gather A, gather B) ----
                for ec in range(NCH):
                    lo = slice(ec * CH, (ec + 1) * CH)
                    nc.gpsimd.ap_gather(
                        scratch[:, 0:CH] if ec == 0 else scanbuf[:, CH:],
                        r_full[:],
                        idxA[:, ec * CH // 16 : (ec + 1) * CH // 16],
                        channels=128, num_elems=SLICE, d=1, num_idxs=CH)
                    nc.gpsimd.ap_gather(
                        scratch[:, CH:], r_full[:],
                        idxB[:, ec * CH // 16 : (ec + 1) * CH // 16],
                        channels=128, num_elems=SLICE, d=1, num_idxs=CH)
                    if ec == 0:
                        # merge B into A (odd partitions), weight, scan
                        nc.vector.copy_predicated(scratch[:, 0:CH], maskCH,
                                                  scratch[:, CH:])
                        nc.vector.tensor_mul(scratch[:, 0:CH],
                                             scratch[:, 0:CH], wq[:, lo])
                        nc.vector.tensor_tensor_scan(
                            scanbuf[:, 0:CH], scratch[:, 0:CH],
                            scratch[:, 0:CH], 0.0,
                            op0=OP.add, op1=OP.bypass)
                    else:
                        nc.vector.copy_predicated(scanbuf[:, CH:], maskCH,
                                                  scratch[:, CH:])
                        nc.vector.tensor_mul(scanbuf[:, CH:],
                                             scanbuf[:, CH:], wq[:, lo])
                        nc.vector.tensor_tensor_scan(
                            scanbuf[:, CH:], scanbuf[:, CH:],
                            scanbuf[:, CH:],
                            scanbuf[:, CH - 1 : CH],
                            op0=OP.add, op1=OP.bypass)

                # ---- boundary sampling: 2 gathers of BOUND_T ----
                nc.gpsimd.ap_gather(
                    scratch[:, 0:BOUND_T], scanbuf[:],
                    bidxA[:], channels=128, num_elems=STREAM, d=1,
                    num_idxs=BOUND_T)
                nc.gpsimd.ap_gather(
                    scratch[:, CH : CH + BOUND_T], scanbuf[:],
                    bidxB[:], channels=128, num_elems=STREAM, d=1,
                    num_idxs=BOUND_T)
                nc.vector.copy_predicated(scratch[:, 0:BOUND_T], maskBT,
                                          scratch[:, CH : CH + BOUND_T])
                # adjacent difference -> per-target partials at cols 0..6400
                nc.vector.tensor_tensor(
                    out=scratch[:, 0:CORE_PAD],
                    in0=scratch[:, 1 : CORE_PAD + 1],
                    in1=scratch[:, 0:CORE_PAD],
                    op=OP.subtract,
                )

                # ---- fused merge+transpose: psum2[:, q*8:+8] =
                # diff[:, q*128:+128]^T x sel  (sums the 16 (g,s) partials
                # per batch and lands directly in the (q b) update layout)
                psum2 = psum_pool.tile([128, UPD_COLS], dt.float32,
                                       space="PSUM", tag="upd")
                for q in range(NQ):
                    nc.tensor.matmul(
                        psum2[:, q * 8 : (q + 1) * 8],
                        scratch[:, q * 128 : (q + 1) * 128],
                        sel[:], start=True, stop=True)

                # ---- update ----
                nc.vector.tensor_tensor(t1[:], psum2[:], xt[t % 2][:],
                                        op=OP.add)
                nc.vector.tensor_mul(v[:], v[:], Ad[:])
                nc.vector.tensor_add(v[:], v[:], t1[:])
                nc.sync.dma_start(out_d[t], v[:])

    nc.compile()
    return nc


def _get_nc(T_steps):
    key = ("nc", T_steps)
    if key not in _CACHE:
        _CACHE[key] = _build(T_steps)
    return _CACHE[key]


def kernel(x, bias, time_const, sign, syn_count, syn_strength,
           source_idx, target_idx):
    from concourse.bass_utils import run_bass_kernel_spmd

    x = np.asarray(x, dtype=np.float32)
    bias = np.asarray(bias, dtype=np.float32)
    time_const = np.asarray(time_const, dtype=np.float32)
    sign = np.asarray(sign, dtype=np.float32)
    syn_count = np.asarray(syn_count, dtype=np.float32)
    syn_strength = np.asarray(syn_strength, dtype=np.float32)
    T_steps = x.shape[0]

    per_core = _preprocess(x, bias, time_const, sign, syn_count,
                           syn_strength, source_idx, target_idx)
    nc = _get_nc(T_steps)
    t0 = time.perf_counter()
    res = run_bass_kernel_spmd(nc, per_core, core_ids=list(range(NC)))
    t1 = time.perf_counter()
    print(f"[kernel] run_bass_kernel_spmd wall: {t1 - t0:.3f}s",
          file=sys.stderr)

    out = np.empty((T_steps, B, N_NODES), dtype=np.float32)
    for c in range(NC):
        vs = res.results[c]["vs"]  # [T, 128, 400], node = q*128+p
        vbn = (vs.reshape(T_steps, 128, NQ, B).transpose(0, 3, 2, 1)
               .reshape(T_steps, B, CORE_PAD))
        out[:, :, c * CORE_REAL : (c + 1) * CORE_REAL] = vbn[:, :, :CORE_REAL]
    return out
